# revision 1
# baseline (speedup 1.0000x reference)
"""Trainium2 Bass kernel for ContrastiveAffinityLossWithMemory.

Strategy (B=4096, D=512, C=4096, dd=384, 8 cores):
  - Host: closed-form of the sequential scatter-EMA memory update (it only
    feeds the loss through the normalized bank m and weights w), gather of
    lookup rows by label pre-scaled to u = w_c*(1-t), and the analytic
    pieces sum(w*d^2) = 2W - 2 x.s_m and sum(d^2) over pairs.
  - Device (SPMD, data-parallel over batch rows, 512 rows/core): the two
    O(B^2 d)/O(B C d) matmuls in bf16 on PE, d = sqrt(2-2cos) on ScalarE
    with fused free-axis accumulation, and sum_c u*d via one fused
    tensor_tensor_reduce on VectorE. Operands are pre-scaled by 0.996 so
    2-2cos stays strictly positive (sqrt-safe); the induced error on the
    final scalar is ~1e-6 relative.
  - Host: combine per-core partial sums (f64) into the final scalar.
"""
import numpy as np
import ml_dtypes

ALPHA = 0.7
DECAY = 0.01
CUR_TIME = 1.0
EPS = 1e-12
MARGIN = 4.0
B, D, C = 4096, 512, 4096
DD = 384
N_CORES = 8
RPC = B // N_CORES          # rows per core = 512
RB = RPC // 128             # row blocks per core = 4
CT_B = B // 512             # col tiles, batch side = 8
CT_M = C // 512             # col tiles, mem side = 8
KC = DD // 128              # contraction chunks = 3
SCALE = 0.996               # operand pre-scale; keeps device cos < 1

TRACE = False               # test harness may flip these
LAST_RESULTS = {}

_NC_CACHE = {}


# ---------------------------------------------------------------- host math
def _l2norm(a):
    n = np.maximum(np.linalg.norm(a, axis=-1, keepdims=True), EPS)
    return (a / n).astype(np.float32)


def _bank_update(l, yp, mem_embeddings, mem_timestamps, mem_initialized):
    """Closed form of the per-sample conditional scatter-EMA over valid
    samples (l already filtered/clipped to [0, C))."""
    Cc, dd = mem_embeddings.shape
    n = l.shape[0]
    init0 = mem_initialized.astype(bool)

    counts = np.bincount(l, minlength=Cc)
    if n:
        order = np.argsort(l, kind="stable")
        ls = l[order]
        grp_start = np.r_[0, np.flatnonzero(np.diff(ls)) + 1]
        start_of_grp = np.repeat(grp_start, np.diff(np.r_[grp_start, n]))
        rank_sorted = np.arange(n) - start_of_grp
        k_i = counts[ls]
        pw = (1.0 - ALPHA) ** (k_i - 1 - rank_sorted).astype(np.float64)
        coef = ALPHA * pw
        first_uninit = (rank_sorted == 0) & (~init0[ls])
        coef[first_uninit] = pw[first_uninit]
        contrib = coef[:, None].astype(np.float32) * yp[order]
        seg = np.add.reduceat(contrib, grp_start, axis=0)
        acc = np.zeros((Cc, dd), dtype=np.float32)
        acc[ls[grp_start]] = seg
    else:
        acc = np.zeros((Cc, dd), dtype=np.float32)

    hit = counts > 0
    coef_old = np.where(hit, np.where(init0, (1.0 - ALPHA) ** counts, 0.0),
                        1.0).astype(np.float32)
    emb_new = coef_old[:, None] * mem_embeddings + acc
    init_new = init0 | hit
    ts_new = np.where(hit, np.float32(CUR_TIME),
                      mem_timestamps).astype(np.float32)
    return emb_new, init_new, ts_new


def _numpy_fallback(y_true, y_pred, lookup, mem_embeddings, mem_timestamps,
                    mem_initialized):
    """Faithful numpy port of the reference; used only if the inputs violate
    the fast path's assumptions (e.g. -1/background labels)."""
    b = y_pred.shape[0]
    c = lookup.shape[0]
    dd = int(y_pred.shape[1] * 0.75)
    yp = y_pred[:, :dd].astype(np.float32)
    l = np.asarray(y_true).astype(np.int64)
    valid = (l >= 0) & (l < c)
    lc = np.clip(l, 0, c - 1)

    emb, init, ts = _bank_update(lc[valid], yp[valid], mem_embeddings,
                                 mem_timestamps, mem_initialized)
    x = _l2norm(yp)
    cos = x @ x.T
    sqd = np.clip(2.0 - 2.0 * cos, 0.0, None)
    tri = np.triu(np.ones((b, b), bool), k=1)
    dist = np.sqrt(np.where(tri, sqd, 1.0))
    is_bg = l == -1
    both = is_bg[:, None] & is_bg[None, :]
    one = is_bg[:, None] ^ is_bg[None, :]
    tsim = np.where(both, 0.2, np.where(one, 0.01, 0.0))
    md = np.maximum(MARGIN - dist, 0.0)
    pair = tsim * dist**2 + (1.0 - tsim) * md**2
    n_pairs = b * (b - 1) // 2
    batch_loss = np.where(tri, pair, 0.0).sum(dtype=np.float64) / n_pairs

    m = np.where(init[:, None], _l2norm(emb), 0.0).astype(np.float32)
    cos_m = x @ m.T
    sqd_m = np.clip(2.0 - 2.0 * cos_m, 0.0, None)
    dist_m = np.sqrt(np.maximum(sqd_m, EPS))
    tsim_m = lookup[lc]
    w = (np.exp(-DECAY * (CUR_TIME - ts)) * init).astype(np.float32)
    md_m = np.maximum(MARGIN - dist_m, 0.0)
    term = (tsim_m * dist_m**2 + (1.0 - tsim_m) * md_m**2) * w[None, :]
    n_init = max(int(init.sum()), 1)
    per_sample = np.where(init[None, :], term, 0.0).sum(
        axis=1, dtype=np.float64) / n_init
    n_valid = max(int(valid.sum()), 1)
    mem_loss = (per_sample * valid).sum(dtype=np.float64) / n_valid
    return np.float32(0.7 * batch_loss + 0.3 * mem_loss)


def _host_prep(y_true, y_pred, lookup, mem_embeddings, mem_timestamps,
               mem_initialized):
    bf16 = ml_dtypes.bfloat16
    l = np.asarray(y_true).astype(np.int64)
    yp = np.ascontiguousarray(y_pred[:, :DD]).astype(np.float32)

    emb, init, ts = _bank_update(l, yp, mem_embeddings, mem_timestamps,
                                 mem_initialized)
    m = np.where(init[:, None], _l2norm(emb), 0.0).astype(np.float32)
    w = (np.exp(-DECAY * (CUR_TIME - ts)) * init).astype(np.float32)
    n_init = max(int(init.sum()), 1)

    x = _l2norm(yp)
    xs = (x * SCALE).astype(bf16)             # [B, DD]
    ms = (m * SCALE).astype(bf16)             # [C, DD]

    t = lookup[l]                             # [B, C] f32 host gather
    w64 = w.astype(np.float64)
    u32 = w[None, :] * (1.0 - t)                                   # [B, C]
    R = u32.sum(axis=1, dtype=np.float64)                          # [B]
    u16 = u32.astype(np.float16)

    xt3 = np.ascontiguousarray(xs.T).reshape(KC, 128, B)
    mt3 = np.ascontiguousarray(ms.T).reshape(KC, 128, C)

    in_maps = []
    for k in range(N_CORES):
        rows = slice(k * RPC, (k + 1) * RPC)
        in_maps.append({
            "xt3": xt3,
            "mt3": mt3,
            "xtk3": np.ascontiguousarray(xs[rows].T).reshape(KC, 128, RPC),
            "u": np.ascontiguousarray(u16[rows]),
        })

    # analytic pieces (f64)
    xs64 = xs.astype(np.float64)
    cos_ii = (xs64 * xs64).sum(axis=1)
    Sd_diag = np.sqrt(np.maximum(2.0 - 2.0 * cos_ii, 0.0)).sum()
    s_vec = xs64.sum(axis=0)
    T2_upper = (B * (B - 1) // 2) * 2.0 - (s_vec @ s_vec - cos_ii.sum())

    W = w64.sum()
    s_m = (w64[:, None] * m.astype(np.float64)).sum(axis=0)
    xdots = x.astype(np.float64) @ s_m

    meta = dict(Sd_diag=Sd_diag, T2_upper=T2_upper, W=W, xdots=xdots, R=R,
                n_init=n_init, n_valid=B)
    return in_maps, meta


def _assemble(results, meta):
    S_all_d = 0.0
    q = np.zeros(B, dtype=np.float64)
    for k, res in enumerate(results):
        s_acc = np.asarray(res["s_acc"], dtype=np.float64)   # [128, 32]
        q_acc = np.asarray(res["q_acc"], dtype=np.float64)   # [128, 32]
        S_all_d += s_acc.sum()
        qk = q_acc.reshape(128, RB, CT_M).sum(axis=2)        # [128, RB]
        for rb in range(RB):
            rows = slice(k * RPC + rb * 128, k * RPC + (rb + 1) * 128)
            q[rows] = qk[:, rb]

    n_pairs = B * (B - 1) // 2
    Sd_upper = (S_all_d - meta["Sd_diag"]) / 2.0
    batch_sum = 16.0 * n_pairs - 8.0 * Sd_upper + meta["T2_upper"]
    batch_loss = batch_sum / n_pairs

    per_i = (2.0 * meta["W"] - 2.0 * meta["xdots"]) + 16.0 * meta["R"] - 8.0 * q
    mem_loss = per_i.sum() / meta["n_init"] / meta["n_valid"]
    return np.float32(0.7 * batch_loss + 0.3 * mem_loss)


# ---------------------------------------------------------------- device
def _build_nc():
    if "nc" in _NC_CACHE:
        return _NC_CACHE["nc"]
    import concourse.bacc as bacc
    import concourse.bass as bass
    import concourse.mybir as mybir
    import concourse.tile as tile
    from concourse._compat import get_trn_type

    f32 = mybir.dt.float32
    bf16 = mybir.dt.bfloat16
    f16 = mybir.dt.float16

    nc = bacc.Bacc(get_trn_type() or "TRN2", target_bir_lowering=False,
                   debug=False)

    xt3 = nc.dram_tensor("xt3", [KC, 128, B], bf16, kind="ExternalInput")
    mt3 = nc.dram_tensor("mt3", [KC, 128, C], bf16, kind="ExternalInput")
    xtk3 = nc.dram_tensor("xtk3", [KC, 128, RPC], bf16, kind="ExternalInput")
    u = nc.dram_tensor("u", [RPC, C], f16, kind="ExternalInput")
    s_out = nc.dram_tensor("s_acc", [128, RB * CT_B], f32,
                           kind="ExternalOutput")
    q_out = nc.dram_tensor("q_acc", [128, RB * CT_M], f32,
                           kind="ExternalOutput")

    with tile.TileContext(nc) as tc:
        with (
            tc.tile_pool(name="const", bufs=1) as const,
            tc.tile_pool(name="psum", bufs=6, space="PSUM") as psum,
            tc.tile_pool(name="work", bufs=4) as work,
        ):
            xall = []
            mall = []
            xk = []
            for kc in range(KC):
                ta = const.tile([128, B], bf16, tag=f"xall{kc}")
                nc.sync.dma_start(ta[:], xt3[kc])
                xall.append(ta)
                tm = const.tile([128, C], bf16, tag=f"mall{kc}")
                nc.sync.dma_start(tm[:], mt3[kc])
                mall.append(tm)
                tk = const.tile([128, RPC], bf16, tag=f"xk{kc}")
                nc.sync.dma_start(tk[:], xtk3[kc])
                xk.append(tk)

            s_acc = const.tile([128, RB * CT_B], f32, tag="s_acc")
            q_acc = const.tile([128, RB * CT_M], f32, tag="q_acc")
            bias2 = const.tile([128, 1], f32, tag="bias2")
            nc.vector.memset(bias2[:], 2.0)

            for rb in range(RB):
                rsl = slice(rb * 128, (rb + 1) * 128)
                for ct in range(CT_B):
                    csl = slice(ct * 512, (ct + 1) * 512)
                    ps = psum.tile([128, 512], f32, tag="ps")
                    for kc in range(KC):
                        nc.tensor.matmul(ps[:], xk[kc][:, rsl],
                                         xall[kc][:, csl],
                                         start=(kc == 0), stop=(kc == KC - 1))
                    col = rb * CT_B + ct
                    db = work.tile([128, 512], bf16, tag="db")
                    nc.scalar.activation(
                        db[:], ps[:], mybir.ActivationFunctionType.Sqrt,
                        bias=bias2[:], scale=-2.0,
                        accum_out=s_acc[:, col:col + 1])
                for ct in range(CT_M):
                    csl = slice(ct * 512, (ct + 1) * 512)
                    ps = psum.tile([128, 512], f32, tag="ps")
                    for kc in range(KC):
                        nc.tensor.matmul(ps[:], xk[kc][:, rsl],
                                         mall[kc][:, csl],
                                         start=(kc == 0), stop=(kc == KC - 1))
                    dm = work.tile([128, 512], f32, tag="dm")
                    nc.scalar.activation(
                        dm[:], ps[:], mybir.ActivationFunctionType.Sqrt,
                        bias=bias2[:], scale=-2.0)
                    ut = work.tile([128, 512], f16, tag="ut")
                    nc.sync.dma_start(ut[:], u[rsl, csl])
                    junk = work.tile([128, 512], f32, tag="junk")
                    col = rb * CT_M + ct
                    nc.vector.tensor_tensor(junk[:], dm[:], ut[:],
                                            mybir.AluOpType.mult)
                    nc.vector.tensor_reduce(q_acc[:, col:col + 1], junk[:],
                                            mybir.AxisListType.XYZW,
                                            mybir.AluOpType.add)

            nc.sync.dma_start(s_out[:], s_acc[:])
            nc.sync.dma_start(q_out[:], q_acc[:])

    nc.compile()
    _NC_CACHE["nc"] = nc
    return nc


def kernel(y_true, y_pred, lookup, mem_embeddings, mem_timestamps,
           mem_initialized):
    y_true = np.asarray(y_true)
    y_pred = np.asarray(y_pred, dtype=np.float32)
    lookup = np.asarray(lookup, dtype=np.float32)
    mem_embeddings = np.asarray(mem_embeddings, dtype=np.float32)
    mem_timestamps = np.asarray(mem_timestamps, dtype=np.float32)
    mem_initialized = np.asarray(mem_initialized, dtype=np.int32)

    l = y_true.astype(np.int64)
    if (y_pred.shape != (B, D) or lookup.shape != (C, C)
            or not ((l >= 0) & (l < C)).all()):
        return _numpy_fallback(y_true, y_pred, lookup, mem_embeddings,
                               mem_timestamps, mem_initialized)

    from concourse.bass_utils import run_bass_kernel_spmd

    nc = _build_nc()
    in_maps, meta = _host_prep(y_true, y_pred, lookup, mem_embeddings,
                               mem_timestamps, mem_initialized)
    res = run_bass_kernel_spmd(nc, in_maps, list(range(N_CORES)),
                               trace=TRACE)
    LAST_RESULTS["bass"] = res
    return _assemble(res.results, meta)



# revision 3
# speedup vs baseline: 18.1037x; 18.1037x over previous
"""Trainium2 Bass kernel for ContrastiveAffinityLossWithMemory.

Strategy (B=4096, D=512, C=4096, dd=384, 8 cores):
  - Host: closed-form of the sequential scatter-EMA memory update (it only
    feeds the loss through the normalized bank m and weights w), gather of
    lookup rows by label pre-scaled to u = w_c*(1-t), and the analytic
    pieces sum(w*d^2) = 2W - 2 x.s_m and sum(d^2) over pairs.
  - Device (SPMD, data-parallel over batch rows, 512 rows/core): the two
    O(B^2 d)/O(B C d) matmuls in bf16 on PE, d = sqrt(2-2cos) on ScalarE
    with fused free-axis accumulation, and sum_c u*d via tensor_tensor +
    tensor_reduce on VectorE. Operands are pre-scaled by 0.996 so
    2-2cos stays strictly positive (sqrt-safe); the induced error on the
    final scalar is ~1e-5 relative.
  - All device inputs are packed into ONE bf16 DRAM tensor per core
    (x^T chunks | m^T chunks | stationary x^T chunks | u row-blocks) moved
    by a single DMA, and both accumulators leave in ONE [128, 64] output.
    A dispatch through the axon tunnel pays a large per-buffer cost, so
    minimizing the I/O arity dominates every on-device consideration.
  - Host: combine per-core partial sums (f64) into the final scalar.
"""
import numpy as np
import ml_dtypes

ALPHA = 0.7
DECAY = 0.01
CUR_TIME = 1.0
EPS = 1e-12
MARGIN = 4.0
B, D, C = 4096, 512, 4096
DD = 384
N_CORES = 8
RPC = B // N_CORES          # rows per core = 512
RB = RPC // 128             # row blocks per core = 4
CT_B = B // 512             # col tiles, batch side = 8
CT_M = C // 512             # col tiles, mem side = 8
KC = DD // 128              # contraction chunks = 3
SCALE = 0.996               # operand pre-scale; keeps device cos < 1

# packed-blob column offsets (bf16 columns)
XOFF = 0                    # x^T, KC chunks of [128, B]
MOFF = XOFF + KC * B        # m^T, KC chunks of [128, C]
KOFF = MOFF + KC * C        # stationary x^T (this core's rows), KC x [128, RPC]
UOFF = KOFF + KC * RPC      # u, RB row-blocks of [128, C]
NCOL = UOFF + RB * C        # = 42496

TRACE = False               # test harness may flip these
LAST_RESULTS = {}

_NC_CACHE = {}


# ---------------------------------------------------------------- host math
def _l2norm(a):
    n = np.maximum(np.linalg.norm(a, axis=-1, keepdims=True), EPS)
    return (a / n).astype(np.float32)


def _bank_update(l, yp, mem_embeddings, mem_timestamps, mem_initialized):
    """Closed form of the per-sample conditional scatter-EMA over valid
    samples (l already filtered/clipped to [0, C))."""
    Cc, dd = mem_embeddings.shape
    n = l.shape[0]
    init0 = mem_initialized.astype(bool)

    counts = np.bincount(l, minlength=Cc)
    if n:
        order = np.argsort(l, kind="stable")
        ls = l[order]
        grp_start = np.r_[0, np.flatnonzero(np.diff(ls)) + 1]
        start_of_grp = np.repeat(grp_start, np.diff(np.r_[grp_start, n]))
        rank_sorted = np.arange(n) - start_of_grp
        k_i = counts[ls]
        pw = (1.0 - ALPHA) ** (k_i - 1 - rank_sorted).astype(np.float64)
        coef = ALPHA * pw
        first_uninit = (rank_sorted == 0) & (~init0[ls])
        coef[first_uninit] = pw[first_uninit]
        contrib = coef[:, None].astype(np.float32) * yp[order]
        seg = np.add.reduceat(contrib, grp_start, axis=0)
        acc = np.zeros((Cc, dd), dtype=np.float32)
        acc[ls[grp_start]] = seg
    else:
        acc = np.zeros((Cc, dd), dtype=np.float32)

    hit = counts > 0
    coef_old = np.where(hit, np.where(init0, (1.0 - ALPHA) ** counts, 0.0),
                        1.0).astype(np.float32)
    emb_new = coef_old[:, None] * mem_embeddings + acc
    init_new = init0 | hit
    ts_new = np.where(hit, np.float32(CUR_TIME),
                      mem_timestamps).astype(np.float32)
    return emb_new, init_new, ts_new


def _numpy_fallback(y_true, y_pred, lookup, mem_embeddings, mem_timestamps,
                    mem_initialized):
    """Faithful numpy port of the reference; used only if the inputs violate
    the fast path's assumptions (e.g. -1/background labels)."""
    b = y_pred.shape[0]
    c = lookup.shape[0]
    dd = int(y_pred.shape[1] * 0.75)
    yp = y_pred[:, :dd].astype(np.float32)
    l = np.asarray(y_true).astype(np.int64)
    valid = (l >= 0) & (l < c)
    lc = np.clip(l, 0, c - 1)

    emb, init, ts = _bank_update(lc[valid], yp[valid], mem_embeddings,
                                 mem_timestamps, mem_initialized)
    x = _l2norm(yp)
    cos = x @ x.T
    sqd = np.clip(2.0 - 2.0 * cos, 0.0, None)
    tri = np.triu(np.ones((b, b), bool), k=1)
    dist = np.sqrt(np.where(tri, sqd, 1.0))
    is_bg = l == -1
    both = is_bg[:, None] & is_bg[None, :]
    one = is_bg[:, None] ^ is_bg[None, :]
    tsim = np.where(both, 0.2, np.where(one, 0.01, 0.0))
    md = np.maximum(MARGIN - dist, 0.0)
    pair = tsim * dist**2 + (1.0 - tsim) * md**2
    n_pairs = b * (b - 1) // 2
    batch_loss = np.where(tri, pair, 0.0).sum(dtype=np.float64) / n_pairs

    m = np.where(init[:, None], _l2norm(emb), 0.0).astype(np.float32)
    cos_m = x @ m.T
    sqd_m = np.clip(2.0 - 2.0 * cos_m, 0.0, None)
    dist_m = np.sqrt(np.maximum(sqd_m, EPS))
    tsim_m = lookup[lc]
    w = (np.exp(-DECAY * (CUR_TIME - ts)) * init).astype(np.float32)
    md_m = np.maximum(MARGIN - dist_m, 0.0)
    term = (tsim_m * dist_m**2 + (1.0 - tsim_m) * md_m**2) * w[None, :]
    n_init = max(int(init.sum()), 1)
    per_sample = np.where(init[None, :], term, 0.0).sum(
        axis=1, dtype=np.float64) / n_init
    n_valid = max(int(valid.sum()), 1)
    mem_loss = (per_sample * valid).sum(dtype=np.float64) / n_valid
    return np.float32(0.7 * batch_loss + 0.3 * mem_loss)


def _host_prep(y_true, y_pred, lookup, mem_embeddings, mem_timestamps,
               mem_initialized):
    bf16 = ml_dtypes.bfloat16
    l = np.asarray(y_true).astype(np.int64)
    yp = np.ascontiguousarray(y_pred[:, :DD]).astype(np.float32)

    emb, init, ts = _bank_update(l, yp, mem_embeddings, mem_timestamps,
                                 mem_initialized)
    m = np.where(init[:, None], _l2norm(emb), 0.0).astype(np.float32)
    w = (np.exp(-DECAY * (CUR_TIME - ts)) * init).astype(np.float32)
    n_init = max(int(init.sum()), 1)

    x = _l2norm(yp)
    xs = (x * SCALE).astype(bf16)             # [B, DD]
    ms = (m * SCALE).astype(bf16)             # [C, DD]

    t = lookup[l]                             # [B, C] f32 host gather
    w64 = w.astype(np.float64)
    u32 = w[None, :] * (1.0 - t)                                   # [B, C]
    R = u32.sum(axis=1, dtype=np.float64)                          # [B]
    ub = u32.astype(bf16)

    xsT = np.ascontiguousarray(xs.T)          # [DD, B]
    msT = np.ascontiguousarray(ms.T)          # [DD, C]

    base = np.empty((128, KOFF), dtype=bf16)
    for kc in range(KC):
        base[:, XOFF + kc * B:XOFF + (kc + 1) * B] = \
            xsT[kc * 128:(kc + 1) * 128]
        base[:, MOFF + kc * C:MOFF + (kc + 1) * C] = \
            msT[kc * 128:(kc + 1) * 128]

    in_maps = []
    for k in range(N_CORES):
        rows = slice(k * RPC, (k + 1) * RPC)
        blob = np.empty((128, NCOL), dtype=bf16)
        blob[:, :KOFF] = base
        for kc in range(KC):
            blob[:, KOFF + kc * RPC:KOFF + (kc + 1) * RPC] = \
                xsT[kc * 128:(kc + 1) * 128, rows]
        uk = ub[rows]                          # [RPC, C]
        for rb in range(RB):
            blob[:, UOFF + rb * C:UOFF + (rb + 1) * C] = \
                uk[rb * 128:(rb + 1) * 128]
        in_maps.append({"blob": blob})

    # analytic pieces (f64)
    xs64 = xs.astype(np.float64)
    cos_ii = (xs64 * xs64).sum(axis=1)
    Sd_diag = np.sqrt(np.maximum(2.0 - 2.0 * cos_ii, 0.0)).sum()
    s_vec = xs64.sum(axis=0)
    T2_upper = (B * (B - 1) // 2) * 2.0 - (s_vec @ s_vec - cos_ii.sum())

    W = w64.sum()
    s_m = (w64[:, None] * m.astype(np.float64)).sum(axis=0)
    xdots = x.astype(np.float64) @ s_m

    meta = dict(Sd_diag=Sd_diag, T2_upper=T2_upper, W=W, xdots=xdots, R=R,
                n_init=n_init, n_valid=B)
    return in_maps, meta


def _assemble(results, meta):
    S_all_d = 0.0
    q = np.zeros(B, dtype=np.float64)
    for k, res in enumerate(results):
        out = np.asarray(res["out"], dtype=np.float64)       # [128, 64]
        s_acc = out[:, 0:RB * CT_B]                          # [128, 32]
        q_acc = out[:, RB * CT_B:RB * (CT_B + CT_M)]         # [128, 32]
        S_all_d += s_acc.sum()
        qk = q_acc.reshape(128, RB, CT_M).sum(axis=2)        # [128, RB]
        for rb in range(RB):
            rows = slice(k * RPC + rb * 128, k * RPC + (rb + 1) * 128)
            q[rows] = qk[:, rb]

    n_pairs = B * (B - 1) // 2
    Sd_upper = (S_all_d - meta["Sd_diag"]) / 2.0
    batch_sum = 16.0 * n_pairs - 8.0 * Sd_upper + meta["T2_upper"]
    batch_loss = batch_sum / n_pairs

    per_i = (2.0 * meta["W"] - 2.0 * meta["xdots"]) + 16.0 * meta["R"] - 8.0 * q
    mem_loss = per_i.sum() / meta["n_init"] / meta["n_valid"]
    return np.float32(0.7 * batch_loss + 0.3 * mem_loss)


# ---------------------------------------------------------------- device
def _build_nc(repeat=1, names=("blob", "out")):
    """Build the SPMD NEFF. repeat>1 re-issues {input DMA + full compute}
    that many times back-to-back (numerically meaningless — it exists only
    so a timing harness can measure marginal per-dispatch device time as a
    slope, cancelling the large fixed dispatch overhead)."""
    key = (repeat, names)
    if key in _NC_CACHE:
        return _NC_CACHE[key]
    import concourse.bacc as bacc
    import concourse.bass as bass
    import concourse.mybir as mybir
    import concourse.tile as tile
    from concourse._compat import get_trn_type

    f32 = mybir.dt.float32
    bf16 = mybir.dt.bfloat16

    nc = bacc.Bacc(get_trn_type() or "TRN2", target_bir_lowering=False,
                   debug=False)

    blob = nc.dram_tensor(names[0], [128, NCOL], bf16, kind="ExternalInput")
    out = nc.dram_tensor(names[1], [128, RB * (CT_B + CT_M)], f32,
                         kind="ExternalOutput")

    with tile.TileContext(nc) as tc:
        with (
            tc.tile_pool(name="const", bufs=1) as const,
            tc.tile_pool(name="psum", bufs=6, space="PSUM") as psum,
            tc.tile_pool(name="work", bufs=4) as work,
        ):
            sb = const.tile([128, NCOL], bf16, tag="blob")
            acc = const.tile([128, RB * (CT_B + CT_M)], f32, tag="acc")
            bias2 = const.tile([128, 1], f32, tag="bias2")
            nc.vector.memset(bias2[:], 2.0)

            for _rep in range(repeat):
                nc.sync.dma_start(sb[:], blob[:])
                for rb in range(RB):
                    for ct in range(CT_B):
                        ps = psum.tile([128, 512], f32, tag="ps")
                        for kc in range(KC):
                            st = KOFF + kc * RPC + rb * 128
                            mv = XOFF + kc * B + ct * 512
                            nc.tensor.matmul(ps[:], sb[:, st:st + 128],
                                             sb[:, mv:mv + 512],
                                             start=(kc == 0),
                                             stop=(kc == KC - 1))
                        col = rb * CT_B + ct
                        db = work.tile([128, 512], bf16, tag="db")
                        nc.scalar.activation(
                            db[:], ps[:], mybir.ActivationFunctionType.Sqrt,
                            bias=bias2[:], scale=-2.0,
                            accum_out=acc[:, col:col + 1])
                    for ct in range(CT_M):
                        ps = psum.tile([128, 512], f32, tag="ps")
                        for kc in range(KC):
                            st = KOFF + kc * RPC + rb * 128
                            mv = MOFF + kc * C + ct * 512
                            nc.tensor.matmul(ps[:], sb[:, st:st + 128],
                                             sb[:, mv:mv + 512],
                                             start=(kc == 0),
                                             stop=(kc == KC - 1))
                        dm = work.tile([128, 512], f32, tag="dm")
                        nc.scalar.activation(
                            dm[:], ps[:], mybir.ActivationFunctionType.Sqrt,
                            bias=bias2[:], scale=-2.0)
                        junk = work.tile([128, 512], f32, tag="junk")
                        col = RB * CT_B + rb * CT_M + ct
                        uc = UOFF + rb * C + ct * 512
                        nc.vector.tensor_tensor(junk[:], dm[:],
                                                sb[:, uc:uc + 512],
                                                mybir.AluOpType.mult)
                        nc.vector.tensor_reduce(acc[:, col:col + 1], junk[:],
                                                mybir.AxisListType.XYZW,
                                                mybir.AluOpType.add)

            nc.sync.dma_start(out[:], acc[:])

    nc.compile()
    _NC_CACHE[key] = nc
    return nc


def kernel(y_true, y_pred, lookup, mem_embeddings, mem_timestamps,
           mem_initialized):
    y_true = np.asarray(y_true)
    y_pred = np.asarray(y_pred, dtype=np.float32)
    lookup = np.asarray(lookup, dtype=np.float32)
    mem_embeddings = np.asarray(mem_embeddings, dtype=np.float32)
    mem_timestamps = np.asarray(mem_timestamps, dtype=np.float32)
    mem_initialized = np.asarray(mem_initialized, dtype=np.int32)

    l = y_true.astype(np.int64)
    if (y_pred.shape != (B, D) or lookup.shape != (C, C)
            or not ((l >= 0) & (l < C)).all()):
        return _numpy_fallback(y_true, y_pred, lookup, mem_embeddings,
                               mem_timestamps, mem_initialized)

    from concourse.bass_utils import run_bass_kernel_spmd

    nc = _build_nc()
    in_maps, meta = _host_prep(y_true, y_pred, lookup, mem_embeddings,
                               mem_timestamps, mem_initialized)
    res = run_bass_kernel_spmd(nc, in_maps, list(range(N_CORES)),
                               trace=TRACE)
    LAST_RESULTS["bass"] = res
    return _assemble(res.results, meta)


# revision 5
# speedup vs baseline: 139.3818x; 7.6991x over previous
"""Trainium2 Bass kernel for ContrastiveAffinityLossWithMemory.

Strategy (B=4096, D=512, C=4096, dd=384, 8 cores):
  - Host: closed-form of the sequential scatter-EMA memory update (it only
    feeds the loss through the normalized bank m and weights w), gather of
    lookup rows by label pre-scaled to u = w_c*(1-t), and the analytic
    pieces sum(w*d^2) = 2W - 2 x.s_m and sum(d^2) over pairs.
  - Device (SPMD, data-parallel over batch rows, 512 rows/core): the two
    O(B^2 d)/O(B C d) matmuls in bf16 on PE, d = sqrt(2-2cos) on ScalarE
    with fused free-axis accumulation, and sum_c u*d via tensor_tensor +
    tensor_reduce on VectorE. Operands are pre-scaled by 0.996 so
    2-2cos stays strictly positive (sqrt-safe); the induced error on the
    final scalar is ~1e-5 relative.
  - All device inputs are packed into ONE bf16 DRAM tensor per core
    (x^T chunks | m^T chunks | stationary x^T chunks | u row-blocks) moved
    by a single DMA, and both accumulators leave in ONE [128, 64] output.
    A dispatch through the axon tunnel pays a large per-buffer cost, so
    minimizing the I/O arity dominates every on-device consideration.
  - Host: combine per-core partial sums (f64) into the final scalar.
"""
import numpy as np
import ml_dtypes

ALPHA = 0.7
DECAY = 0.01
CUR_TIME = 1.0
EPS = 1e-12
MARGIN = 4.0
B, D, C = 4096, 512, 4096
DD = 384
N_CORES = 8
RPC = B // N_CORES          # rows per core = 512
RB = RPC // 128             # row blocks per core = 4
CT_B = B // 512             # col tiles, batch side = 8
CT_M = C // 512             # col tiles, mem side = 8
KC = DD // 128              # contraction chunks = 3
SCALE = 0.996               # operand pre-scale; keeps device cos < 1

# packed-blob column offsets (bf16 columns)
XOFF = 0                    # x^T, KC chunks of [128, B]
MOFF = XOFF + KC * B        # m^T, KC chunks of [128, C]
KOFF = MOFF + KC * C        # stationary x^T (this core's rows), KC x [128, RPC]
UOFF = KOFF + KC * RPC      # u, RB row-blocks of [128, C]
NCOL = UOFF + RB * C        # = 42496

TRACE = False               # test harness may flip these
LAST_RESULTS = {}

_NC_CACHE = {}


# ---------------------------------------------------------------- host math
def _l2norm(a):
    n = np.maximum(np.linalg.norm(a, axis=-1, keepdims=True), EPS)
    return (a / n).astype(np.float32)


def _bank_update(l, yp, mem_embeddings, mem_timestamps, mem_initialized):
    """Closed form of the per-sample conditional scatter-EMA over valid
    samples (l already filtered/clipped to [0, C))."""
    Cc, dd = mem_embeddings.shape
    n = l.shape[0]
    init0 = mem_initialized.astype(bool)

    counts = np.bincount(l, minlength=Cc)
    if n:
        order = np.argsort(l, kind="stable")
        ls = l[order]
        grp_start = np.r_[0, np.flatnonzero(np.diff(ls)) + 1]
        start_of_grp = np.repeat(grp_start, np.diff(np.r_[grp_start, n]))
        rank_sorted = np.arange(n) - start_of_grp
        k_i = counts[ls]
        pw = (1.0 - ALPHA) ** (k_i - 1 - rank_sorted).astype(np.float64)
        coef = ALPHA * pw
        first_uninit = (rank_sorted == 0) & (~init0[ls])
        coef[first_uninit] = pw[first_uninit]
        contrib = coef[:, None].astype(np.float32) * yp[order]
        seg = np.add.reduceat(contrib, grp_start, axis=0)
        acc = np.zeros((Cc, dd), dtype=np.float32)
        acc[ls[grp_start]] = seg
    else:
        acc = np.zeros((Cc, dd), dtype=np.float32)

    hit = counts > 0
    coef_old = np.where(hit, np.where(init0, (1.0 - ALPHA) ** counts, 0.0),
                        1.0).astype(np.float32)
    emb_new = coef_old[:, None] * mem_embeddings + acc
    init_new = init0 | hit
    ts_new = np.where(hit, np.float32(CUR_TIME),
                      mem_timestamps).astype(np.float32)
    return emb_new, init_new, ts_new


def _numpy_fallback(y_true, y_pred, lookup, mem_embeddings, mem_timestamps,
                    mem_initialized):
    """Faithful numpy port of the reference; used only if the inputs violate
    the fast path's assumptions (e.g. -1/background labels)."""
    b = y_pred.shape[0]
    c = lookup.shape[0]
    dd = int(y_pred.shape[1] * 0.75)
    yp = y_pred[:, :dd].astype(np.float32)
    l = np.asarray(y_true).astype(np.int64)
    valid = (l >= 0) & (l < c)
    lc = np.clip(l, 0, c - 1)

    emb, init, ts = _bank_update(lc[valid], yp[valid], mem_embeddings,
                                 mem_timestamps, mem_initialized)
    x = _l2norm(yp)
    cos = x @ x.T
    sqd = np.clip(2.0 - 2.0 * cos, 0.0, None)
    tri = np.triu(np.ones((b, b), bool), k=1)
    dist = np.sqrt(np.where(tri, sqd, 1.0))
    is_bg = l == -1
    both = is_bg[:, None] & is_bg[None, :]
    one = is_bg[:, None] ^ is_bg[None, :]
    tsim = np.where(both, 0.2, np.where(one, 0.01, 0.0))
    md = np.maximum(MARGIN - dist, 0.0)
    pair = tsim * dist**2 + (1.0 - tsim) * md**2
    n_pairs = b * (b - 1) // 2
    batch_loss = np.where(tri, pair, 0.0).sum(dtype=np.float64) / n_pairs

    m = np.where(init[:, None], _l2norm(emb), 0.0).astype(np.float32)
    cos_m = x @ m.T
    sqd_m = np.clip(2.0 - 2.0 * cos_m, 0.0, None)
    dist_m = np.sqrt(np.maximum(sqd_m, EPS))
    tsim_m = lookup[lc]
    w = (np.exp(-DECAY * (CUR_TIME - ts)) * init).astype(np.float32)
    md_m = np.maximum(MARGIN - dist_m, 0.0)
    term = (tsim_m * dist_m**2 + (1.0 - tsim_m) * md_m**2) * w[None, :]
    n_init = max(int(init.sum()), 1)
    per_sample = np.where(init[None, :], term, 0.0).sum(
        axis=1, dtype=np.float64) / n_init
    n_valid = max(int(valid.sum()), 1)
    mem_loss = (per_sample * valid).sum(dtype=np.float64) / n_valid
    return np.float32(0.7 * batch_loss + 0.3 * mem_loss)


def _host_prep(y_true, y_pred, lookup, mem_embeddings, mem_timestamps,
               mem_initialized):
    bf16 = ml_dtypes.bfloat16
    l = np.asarray(y_true).astype(np.int64)
    yp = np.ascontiguousarray(y_pred[:, :DD]).astype(np.float32)

    emb, init, ts = _bank_update(l, yp, mem_embeddings, mem_timestamps,
                                 mem_initialized)
    m = np.where(init[:, None], _l2norm(emb), 0.0).astype(np.float32)
    w = (np.exp(-DECAY * (CUR_TIME - ts)) * init).astype(np.float32)
    n_init = max(int(init.sum()), 1)

    x = _l2norm(yp)
    xs = (x * SCALE).astype(bf16)             # [B, DD]
    ms = (m * SCALE).astype(bf16)             # [C, DD]

    t = lookup[l]                             # [B, C] f32 host gather
    w64 = w.astype(np.float64)
    u32 = w[None, :] * (1.0 - t)                                   # [B, C]
    R = u32.sum(axis=1, dtype=np.float64)                          # [B]
    ub = u32.astype(bf16)

    xsT = np.ascontiguousarray(xs.T)          # [DD, B]
    msT = np.ascontiguousarray(ms.T)          # [DD, C]

    base = np.empty((128, KOFF), dtype=bf16)
    for kc in range(KC):
        base[:, XOFF + kc * B:XOFF + (kc + 1) * B] = \
            xsT[kc * 128:(kc + 1) * 128]
        base[:, MOFF + kc * C:MOFF + (kc + 1) * C] = \
            msT[kc * 128:(kc + 1) * 128]

    in_maps = []
    for k in range(N_CORES):
        rows = slice(k * RPC, (k + 1) * RPC)
        blob = np.empty((128, NCOL), dtype=bf16)
        blob[:, :KOFF] = base
        for kc in range(KC):
            blob[:, KOFF + kc * RPC:KOFF + (kc + 1) * RPC] = \
                xsT[kc * 128:(kc + 1) * 128, rows]
        uk = ub[rows]                          # [RPC, C]
        for rb in range(RB):
            blob[:, UOFF + rb * C:UOFF + (rb + 1) * C] = \
                uk[rb * 128:(rb + 1) * 128]
        in_maps.append({"blob": blob})

    # analytic pieces (f64)
    xs64 = xs.astype(np.float64)
    cos_ii = (xs64 * xs64).sum(axis=1)
    Sd_diag = np.sqrt(np.maximum(2.0 - 2.0 * cos_ii, 0.0)).sum()
    s_vec = xs64.sum(axis=0)
    T2_upper = (B * (B - 1) // 2) * 2.0 - (s_vec @ s_vec - cos_ii.sum())

    W = w64.sum()
    s_m = (w64[:, None] * m.astype(np.float64)).sum(axis=0)
    xdots = x.astype(np.float64) @ s_m

    meta = dict(Sd_diag=Sd_diag, T2_upper=T2_upper, W=W, xdots=xdots, R=R,
                n_init=n_init, n_valid=B)
    return in_maps, meta


def _assemble(results, meta):
    S_all_d = 0.0
    q = np.zeros(B, dtype=np.float64)
    for k, res in enumerate(results):
        out = np.asarray(res["out"], dtype=np.float64)       # [128, 16]
        s_acc = out[:, 0:RB * 2]                             # [128, 8]
        q_acc = out[:, RB * 2:RB * 4]                        # [128, 8]
        S_all_d += s_acc.sum()
        qk = q_acc.reshape(128, RB, 2).sum(axis=2)           # [128, RB]
        for rb in range(RB):
            rows = slice(k * RPC + rb * 128, k * RPC + (rb + 1) * 128)
            q[rows] = qk[:, rb]

    n_pairs = B * (B - 1) // 2
    Sd_upper = (S_all_d - meta["Sd_diag"]) / 2.0
    batch_sum = 16.0 * n_pairs - 8.0 * Sd_upper + meta["T2_upper"]
    batch_loss = batch_sum / n_pairs

    per_i = (2.0 * meta["W"] - 2.0 * meta["xdots"]) + 16.0 * meta["R"] - 8.0 * q
    mem_loss = per_i.sum() / meta["n_init"] / meta["n_valid"]
    return np.float32(0.7 * batch_loss + 0.3 * mem_loss)


# ---------------------------------------------------------------- device
def _build_nc(repeat=1, names=("blob", "out")):
    """Build the SPMD NEFF. repeat>1 re-issues {input DMA + full compute}
    that many times back-to-back (numerically meaningless — it exists only
    so a timing harness can measure marginal per-dispatch device time as a
    slope, cancelling the large fixed dispatch overhead)."""
    key = (repeat, names)
    if key in _NC_CACHE:
        return _NC_CACHE[key]
    import concourse.bacc as bacc
    import concourse.bass as bass
    import concourse.mybir as mybir
    import concourse.tile as tile
    from concourse._compat import get_trn_type

    f32 = mybir.dt.float32
    bf16 = mybir.dt.bfloat16

    nc = bacc.Bacc(get_trn_type() or "TRN2", target_bir_lowering=False,
                   debug=False)

    blob = nc.dram_tensor(names[0], [128, NCOL], bf16, kind="ExternalInput")
    out = nc.dram_tensor(names[1], [128, RB * 4], f32, kind="ExternalOutput")

    W2 = 2048                   # activation/DVE tile width (4 PSUM banks)
    with tile.TileContext(nc) as tc:
        with (
            tc.tile_pool(name="const", bufs=1) as const,
            tc.tile_pool(name="psum", bufs=2, space="PSUM") as psum,
            tc.tile_pool(name="work", bufs=3) as work,
        ):
            # separate SBUF tiles per blob section so compute can start as
            # soon as its section has landed (HWDGE DMAs drain in order)
            xk_sb = const.tile([128, KC * RPC], bf16, tag="xk")
            x_sb = const.tile([128, KC * B], bf16, tag="x")
            m_sb = const.tile([128, KC * C], bf16, tag="m")
            u_sb = const.tile([128, RB * C], bf16, tag="u")
            acc = const.tile([128, RB * 4], f32, tag="acc")
            bias2 = const.tile([128, 1], f32, tag="bias2")
            nc.vector.memset(bias2[:], 2.0)

            for _rep in range(repeat):
                nc.sync.dma_start(xk_sb[:], blob[:, KOFF:KOFF + KC * RPC])
                nc.sync.dma_start(x_sb[:], blob[:, XOFF:XOFF + KC * B])
                nc.sync.dma_start(m_sb[:], blob[:, MOFF:MOFF + KC * C])
                nc.sync.dma_start(u_sb[:], blob[:, UOFF:UOFF + RB * C])
                for rb in range(RB):
                    for h in range(B // W2):          # batch side
                        ps = psum.tile([128, W2], f32, tag="ps")
                        for j in range(W2 // 512):
                            for kc in range(KC):
                                st = kc * RPC + rb * 128
                                mv = kc * B + h * W2 + j * 512
                                nc.tensor.matmul(
                                    ps[:, j * 512:(j + 1) * 512],
                                    xk_sb[:, st:st + 128],
                                    x_sb[:, mv:mv + 512],
                                    start=(kc == 0), stop=(kc == KC - 1))
                        col = rb * 2 + h
                        db = work.tile([128, W2], bf16, tag="db")
                        nc.scalar.activation(
                            db[:], ps[:], mybir.ActivationFunctionType.Sqrt,
                            bias=bias2[:], scale=-2.0,
                            accum_out=acc[:, col:col + 1])
                    for h in range(C // W2):          # memory side
                        ps = psum.tile([128, W2], f32, tag="ps")
                        for j in range(W2 // 512):
                            for kc in range(KC):
                                st = kc * RPC + rb * 128
                                mv = kc * C + h * W2 + j * 512
                                nc.tensor.matmul(
                                    ps[:, j * 512:(j + 1) * 512],
                                    xk_sb[:, st:st + 128],
                                    m_sb[:, mv:mv + 512],
                                    start=(kc == 0), stop=(kc == KC - 1))
                        dm = work.tile([128, W2], f32, tag="dm")
                        nc.scalar.activation(
                            dm[:], ps[:], mybir.ActivationFunctionType.Sqrt,
                            bias=bias2[:], scale=-2.0)
                        junk = work.tile([128, W2], f32, tag="junk")
                        col = RB * 2 + rb * 2 + h
                        uc = rb * C + h * W2
                        nc.vector.scalar_tensor_tensor(
                            junk[:], dm[:], 1.0, u_sb[:, uc:uc + W2],
                            op0=mybir.AluOpType.mult,
                            op1=mybir.AluOpType.mult,
                            accum_out=acc[:, col:col + 1])

            nc.sync.dma_start(out[:], acc[:])

    nc.compile()
    _NC_CACHE[key] = nc
    return nc


def kernel(y_true, y_pred, lookup, mem_embeddings, mem_timestamps,
           mem_initialized):
    y_true = np.asarray(y_true)
    y_pred = np.asarray(y_pred, dtype=np.float32)
    lookup = np.asarray(lookup, dtype=np.float32)
    mem_embeddings = np.asarray(mem_embeddings, dtype=np.float32)
    mem_timestamps = np.asarray(mem_timestamps, dtype=np.float32)
    mem_initialized = np.asarray(mem_initialized, dtype=np.int32)

    l = y_true.astype(np.int64)
    if (y_pred.shape != (B, D) or lookup.shape != (C, C)
            or not ((l >= 0) & (l < C)).all()):
        return _numpy_fallback(y_true, y_pred, lookup, mem_embeddings,
                               mem_timestamps, mem_initialized)

    from concourse.bass_utils import run_bass_kernel_spmd

    nc = _build_nc()
    in_maps, meta = _host_prep(y_true, y_pred, lookup, mem_embeddings,
                               mem_timestamps, mem_initialized)
    res = run_bass_kernel_spmd(nc, in_maps, list(range(N_CORES)),
                               trace=TRACE)
    LAST_RESULTS["bass"] = res
    return _assemble(res.results, meta)


# revision 8
# speedup vs baseline: 190.4588x; 1.3665x over previous
"""Trainium2 Bass kernel for ContrastiveAffinityLossWithMemory.

Strategy (B=4096, D=512, C=4096, dd=384, 8 cores):
  - Host: closed-form of the sequential scatter-EMA memory update (it only
    feeds the loss through the normalized bank m and weights w), gather of
    lookup rows by label pre-scaled to u = w_c*(1-t), and the analytic
    pieces sum(w*d^2) = 2W - 2 x.s_m and sum(d^2) over pairs.
  - Device (SPMD, data-parallel over batch rows, 512 rows/core): the two
    O(B^2 d)/O(B C d) matmuls in bf16 on PE, d = sqrt(2-2cos) on ScalarE
    with fused free-axis accumulation, and sum_c u*d via tensor_tensor +
    tensor_reduce on VectorE. Operands are pre-scaled by 0.996 so
    2-2cos stays strictly positive (sqrt-safe); the induced error on the
    final scalar is ~1e-5 relative.
  - All device inputs are packed into ONE bf16 DRAM tensor per core
    (x^T chunks | m^T chunks | stationary x^T chunks | u row-blocks) moved
    by a single DMA, and both accumulators leave in ONE [128, 64] output.
    A dispatch through the axon tunnel pays a large per-buffer cost, so
    minimizing the I/O arity dominates every on-device consideration.
  - Host: combine per-core partial sums (f64) into the final scalar.
"""
import numpy as np
import ml_dtypes

ALPHA = 0.7
DECAY = 0.01
CUR_TIME = 1.0
EPS = 1e-12
MARGIN = 4.0
B, D, C = 4096, 512, 4096
DD = 384
N_CORES = 8
RPC = B // N_CORES          # rows per core = 512
RB = RPC // 128             # row blocks per core = 4
CT_B = B // 512             # col tiles, batch side = 8
CT_M = C // 512             # col tiles, mem side = 8
KC = DD // 128              # contraction chunks = 3
SCALE = 0.992               # operand pre-scale; keeps device cos < 1
PRE = 16.0                  # fp8 pre-scale (power of 2; folded into act scale)

TRACE = False               # test harness may flip these
LAST_RESULTS = {}

_NC_CACHE = {}


# ---------------------------------------------------------------- host math
def _l2norm(a):
    n = np.maximum(np.linalg.norm(a, axis=-1, keepdims=True), EPS)
    return (a / n).astype(np.float32)


def _bank_update(l, yp, mem_embeddings, mem_timestamps, mem_initialized):
    """Closed form of the per-sample conditional scatter-EMA over valid
    samples (l already filtered/clipped to [0, C))."""
    Cc, dd = mem_embeddings.shape
    n = l.shape[0]
    init0 = mem_initialized.astype(bool)

    counts = np.bincount(l, minlength=Cc)
    if n:
        order = np.argsort(l, kind="stable")
        ls = l[order]
        grp_start = np.r_[0, np.flatnonzero(np.diff(ls)) + 1]
        start_of_grp = np.repeat(grp_start, np.diff(np.r_[grp_start, n]))
        rank_sorted = np.arange(n) - start_of_grp
        k_i = counts[ls]
        pw = (1.0 - ALPHA) ** (k_i - 1 - rank_sorted).astype(np.float64)
        coef = ALPHA * pw
        first_uninit = (rank_sorted == 0) & (~init0[ls])
        coef[first_uninit] = pw[first_uninit]
        contrib = coef[:, None].astype(np.float32) * yp[order]
        seg = np.add.reduceat(contrib, grp_start, axis=0)
        acc = np.zeros((Cc, dd), dtype=np.float32)
        acc[ls[grp_start]] = seg
    else:
        acc = np.zeros((Cc, dd), dtype=np.float32)

    hit = counts > 0
    coef_old = np.where(hit, np.where(init0, (1.0 - ALPHA) ** counts, 0.0),
                        1.0).astype(np.float32)
    emb_new = coef_old[:, None] * mem_embeddings + acc
    init_new = init0 | hit
    ts_new = np.where(hit, np.float32(CUR_TIME),
                      mem_timestamps).astype(np.float32)
    return emb_new, init_new, ts_new


def _numpy_fallback(y_true, y_pred, lookup, mem_embeddings, mem_timestamps,
                    mem_initialized):
    """Faithful numpy port of the reference; used only if the inputs violate
    the fast path's assumptions (e.g. -1/background labels)."""
    b = y_pred.shape[0]
    c = lookup.shape[0]
    dd = int(y_pred.shape[1] * 0.75)
    yp = y_pred[:, :dd].astype(np.float32)
    l = np.asarray(y_true).astype(np.int64)
    valid = (l >= 0) & (l < c)
    lc = np.clip(l, 0, c - 1)

    emb, init, ts = _bank_update(lc[valid], yp[valid], mem_embeddings,
                                 mem_timestamps, mem_initialized)
    x = _l2norm(yp)
    cos = x @ x.T
    sqd = np.clip(2.0 - 2.0 * cos, 0.0, None)
    tri = np.triu(np.ones((b, b), bool), k=1)
    dist = np.sqrt(np.where(tri, sqd, 1.0))
    is_bg = l == -1
    both = is_bg[:, None] & is_bg[None, :]
    one = is_bg[:, None] ^ is_bg[None, :]
    tsim = np.where(both, 0.2, np.where(one, 0.01, 0.0))
    md = np.maximum(MARGIN - dist, 0.0)
    pair = tsim * dist**2 + (1.0 - tsim) * md**2
    n_pairs = b * (b - 1) // 2
    batch_loss = np.where(tri, pair, 0.0).sum(dtype=np.float64) / n_pairs

    m = np.where(init[:, None], _l2norm(emb), 0.0).astype(np.float32)
    cos_m = x @ m.T
    sqd_m = np.clip(2.0 - 2.0 * cos_m, 0.0, None)
    dist_m = np.sqrt(np.maximum(sqd_m, EPS))
    tsim_m = lookup[lc]
    w = (np.exp(-DECAY * (CUR_TIME - ts)) * init).astype(np.float32)
    md_m = np.maximum(MARGIN - dist_m, 0.0)
    term = (tsim_m * dist_m**2 + (1.0 - tsim_m) * md_m**2) * w[None, :]
    n_init = max(int(init.sum()), 1)
    per_sample = np.where(init[None, :], term, 0.0).sum(
        axis=1, dtype=np.float64) / n_init
    n_valid = max(int(valid.sum()), 1)
    mem_loss = (per_sample * valid).sum(dtype=np.float64) / n_valid
    return np.float32(0.7 * batch_loss + 0.3 * mem_loss)


def _host_prep(y_true, y_pred, lookup, mem_embeddings, mem_timestamps,
               mem_initialized):
    bf16 = ml_dtypes.bfloat16
    f8 = ml_dtypes.float8_e4m3                # TRN float8e4 bit-compatible
    l = np.asarray(y_true).astype(np.int64)
    yp = np.ascontiguousarray(y_pred[:, :DD]).astype(np.float32)

    emb, init, ts = _bank_update(l, yp, mem_embeddings, mem_timestamps,
                                 mem_initialized)
    m = np.where(init[:, None], _l2norm(emb), 0.0).astype(np.float32)
    w = (np.exp(-DECAY * (CUR_TIME - ts)) * init).astype(np.float32)
    n_init = max(int(init.sum()), 1)

    x = _l2norm(yp)
    xq = (x * (SCALE * PRE)).astype(f8)       # [B, DD]
    mq = (m * (SCALE * PRE)).astype(f8)       # [C, DD]

    t = lookup[l]                             # [B, C] f32 host gather
    w64 = w.astype(np.float64)
    u32 = w[None, :] * (1.0 - t)                                   # [B, C]
    R = u32.sum(axis=1, dtype=np.float64)                          # [B]
    ub = u32.astype(bf16)

    xqT = np.ascontiguousarray(xq.T)          # [DD, B]
    mqT = np.ascontiguousarray(mq.T)          # [DD, C]
    x8 = np.ascontiguousarray(
        xqT.reshape(KC, 128, B).transpose(1, 0, 2))   # [128, KC, B]
    m8 = np.ascontiguousarray(
        mqT.reshape(KC, 128, C).transpose(1, 0, 2))   # [128, KC, C]

    in_maps = []
    for k in range(N_CORES):
        rows = slice(k * RPC, (k + 1) * RPC)
        k8 = np.ascontiguousarray(
            xqT[:, rows].reshape(KC, 128, RPC).transpose(1, 0, 2))
        uk = np.ascontiguousarray(
            ub[rows].reshape(RB, 128, C).transpose(1, 0, 2)
        ).reshape(128, RB * C)
        in_maps.append({"x8": x8, "m8": m8, "k8": k8, "ub": uk})

    # analytic pieces (f64) from the exact values the device will see
    xs_eff = xq.astype(np.float64) / PRE
    cos_ii = (xs_eff * xs_eff).sum(axis=1)
    Sd_diag = np.sqrt(np.maximum(2.0 - 2.0 * cos_ii, 0.0)).sum()
    s_vec = xs_eff.sum(axis=0)
    T2_upper = (B * (B - 1) // 2) * 2.0 - (s_vec @ s_vec - cos_ii.sum())

    W = w64.sum()
    s_m = (w64[:, None] * m.astype(np.float64)).sum(axis=0)
    xdots = x.astype(np.float64) @ s_m

    meta = dict(Sd_diag=Sd_diag, T2_upper=T2_upper, W=W, xdots=xdots, R=R,
                n_init=n_init, n_valid=B)
    return in_maps, meta


def _assemble(results, meta):
    S_all_d = 0.0
    q = np.zeros(B, dtype=np.float64)
    for k, res in enumerate(results):
        out = np.asarray(res["out"], dtype=np.float64)       # [128, 16]
        s_acc = out[:, 0:RB * 2]                             # [128, 8]
        q_acc = out[:, RB * 2:RB * 4]                        # [128, 8]
        S_all_d += s_acc.sum()
        qk = q_acc.reshape(128, RB, 2).sum(axis=2)           # [128, RB]
        for rb in range(RB):
            rows = slice(k * RPC + rb * 128, k * RPC + (rb + 1) * 128)
            q[rows] = qk[:, rb]

    n_pairs = B * (B - 1) // 2
    Sd_upper = (S_all_d - meta["Sd_diag"]) / 2.0
    batch_sum = 16.0 * n_pairs - 8.0 * Sd_upper + meta["T2_upper"]
    batch_loss = batch_sum / n_pairs

    per_i = (2.0 * meta["W"] - 2.0 * meta["xdots"]) + 16.0 * meta["R"] - 8.0 * q
    mem_loss = per_i.sum() / meta["n_init"] / meta["n_valid"]
    return np.float32(0.7 * batch_loss + 0.3 * mem_loss)


# ---------------------------------------------------------------- device
def _build_nc(repeat=1, suffix=""):
    """Build the SPMD NEFF. repeat>1 re-issues {input DMA + full compute}
    that many times back-to-back (numerically meaningless — it exists only
    so a timing harness can measure marginal per-dispatch device time as a
    slope, cancelling the large fixed dispatch overhead)."""
    key = (repeat, suffix)
    if key in _NC_CACHE:
        return _NC_CACHE[key]
    import concourse.bacc as bacc
    import concourse.bass as bass
    import concourse.mybir as mybir
    import concourse.tile as tile
    from concourse._compat import get_trn_type

    f32 = mybir.dt.float32
    bf16 = mybir.dt.bfloat16
    f8 = mybir.dt.float8e4
    DR = mybir.MatmulPerfMode.DoubleRow

    nc = bacc.Bacc(get_trn_type() or "TRN2", target_bir_lowering=False,
                   debug=False)

    x8 = nc.dram_tensor("x8" + suffix, [128, KC, B], f8,
                        kind="ExternalInput")
    m8 = nc.dram_tensor("m8" + suffix, [128, KC, C], f8,
                        kind="ExternalInput")
    k8 = nc.dram_tensor("k8" + suffix, [128, KC, RPC], f8,
                        kind="ExternalInput")
    ubt = nc.dram_tensor("ub" + suffix, [128, RB * C], bf16,
                         kind="ExternalInput")
    out = nc.dram_tensor("out" + suffix, [128, RB * 4], f32,
                         kind="ExternalOutput")

    W2 = 2048                   # activation/DVE tile width (4 PSUM banks)
    ACT_SCALE = -2.0 / (PRE * PRE)
    with tile.TileContext(nc) as tc:
        with (
            tc.tile_pool(name="const", bufs=1) as const,
            tc.tile_pool(name="psum", bufs=2, space="PSUM") as psum,
            tc.tile_pool(name="work", bufs=3) as work,
        ):
            # separate SBUF tiles per input so compute can start as soon as
            # its section has landed (HWDGE DMAs drain in order)
            xk_sb = const.tile([128, KC, RPC], f8, tag="xk")
            x_sb = const.tile([128, KC, B], f8, tag="x")
            m_sb = const.tile([128, KC, C], f8, tag="m")
            u_sb = const.tile([128, RB * C], bf16, tag="u")
            acc = const.tile([128, RB * 4], f32, tag="acc")
            bias2 = const.tile([128, 1], f32, tag="bias2")
            nc.vector.memset(bias2[:], 2.0)

            def gram_tile(ps, rb, side_sb, base, h):
                """accumulate one [128, W2] block of x_k @ side^T into ps:
                one fp8 DoubleRow matmul (k-chunks 0,1) + one normal fp8
                matmul (k-chunk 2) per 512-wide slice."""
                for j in range(W2 // 512):
                    dst = ps[:, j * 512:(j + 1) * 512]
                    mv = base + h * W2 + j * 512
                    nc.tensor.matmul(
                        dst, xk_sb[:, 0:2, rb * 128:rb * 128 + 128],
                        side_sb[:, 0:2, mv:mv + 512],
                        start=True, stop=False, perf_mode=DR)
                    nc.tensor.matmul(
                        dst, xk_sb[:, 2:3, rb * 128:rb * 128 + 128],
                        side_sb[:, 2:3, mv:mv + 512],
                        start=False, stop=True)

            for _rep in range(repeat):
                nc.sync.dma_start(xk_sb[:], k8[:])
                nc.sync.dma_start(x_sb[:], x8[:])
                nc.sync.dma_start(m_sb[:], m8[:])
                nc.sync.dma_start(u_sb[:], ubt[:])
                for rb in range(RB):
                    for h in range(B // W2):          # batch side
                        ps = psum.tile([128, W2], f32, tag="ps")
                        gram_tile(ps, rb, x_sb, 0, h)
                        col = rb * 2 + h
                        db = work.tile([128, W2], bf16, tag="db")
                        nc.scalar.activation(
                            db[:], ps[:], mybir.ActivationFunctionType.Sqrt,
                            bias=bias2[:], scale=ACT_SCALE,
                            accum_out=acc[:, col:col + 1])
                    for h in range(C // W2):          # memory side
                        ps = psum.tile([128, W2], f32, tag="ps")
                        gram_tile(ps, rb, m_sb, 0, h)
                        dm = work.tile([128, W2], f32, tag="dm")
                        nc.scalar.activation(
                            dm[:], ps[:], mybir.ActivationFunctionType.Sqrt,
                            bias=bias2[:], scale=ACT_SCALE)
                        junk = work.tile([128, W2], f32, tag="junk")
                        col = RB * 2 + rb * 2 + h
                        uc = rb * C + h * W2
                        nc.vector.scalar_tensor_tensor(
                            junk[:], dm[:], 1.0, u_sb[:, uc:uc + W2],
                            op0=mybir.AluOpType.mult,
                            op1=mybir.AluOpType.mult,
                            accum_out=acc[:, col:col + 1])

            nc.sync.dma_start(out[:], acc[:])

    nc.compile()
    _NC_CACHE[key] = nc
    return nc


def kernel(y_true, y_pred, lookup, mem_embeddings, mem_timestamps,
           mem_initialized):
    y_true = np.asarray(y_true)
    y_pred = np.asarray(y_pred, dtype=np.float32)
    lookup = np.asarray(lookup, dtype=np.float32)
    mem_embeddings = np.asarray(mem_embeddings, dtype=np.float32)
    mem_timestamps = np.asarray(mem_timestamps, dtype=np.float32)
    mem_initialized = np.asarray(mem_initialized, dtype=np.int32)

    l = y_true.astype(np.int64)
    if (y_pred.shape != (B, D) or lookup.shape != (C, C)
            or not ((l >= 0) & (l < C)).all()):
        return _numpy_fallback(y_true, y_pred, lookup, mem_embeddings,
                               mem_timestamps, mem_initialized)

    from concourse.bass_utils import run_bass_kernel_spmd

    nc = _build_nc()
    in_maps, meta = _host_prep(y_true, y_pred, lookup, mem_embeddings,
                               mem_timestamps, mem_initialized)
    res = run_bass_kernel_spmd(nc, in_maps, list(range(N_CORES)),
                               trace=TRACE)
    LAST_RESULTS["bass"] = res
    return _assemble(res.results, meta)
